# revision 72
# baseline (speedup 1.0000x reference)
"""AttentionOnAttention Trainium2 kernel (8 NeuronCores, SPMD).

Sharding: core c handles batch b = c//4 and heads [4*(c%4), 4*(c%4)+4);
each core computes the disjoint output slice out[b, :, 256*(c%4):...] so no
collectives are needed.

~194.5us (baseline 217us).  The ScalarE exp stream is the pacing engine
(~1.0us per [128,1024] exp, 128 of them); everything else is scheduled
under it instead of in serial phases:

  - k and v projections in fp8e4 DoubleRow (2 k-tiles contracted per pass);
    q stays bf16 (fp8 q pollutes I/G directly: rel err 4e-2 vs 5e-3).
    Inputs land partition-major/chunk-major so DMAs are few and wide
    (each dma_start costs ~0.6us sync-engine issue; one queue ~100GB/s).
  - Dense prefix (k-pair0 all chunks, q-pair0 chunk0, v0-7) keeps HAM warm;
    pair-1 projections, v8-15 and the finals run as ~120ns micro-closure
    fillers popped between attention steps (whole-unit pops stall the exp
    stream; deadline units are force-drained via need()).
  - PV consumes es in fp8 DoubleRow j-tile PAIRS (stationary v_aug
    [128,2,65], moving es [128,2,512]) lagging the exp stream by LAGP=2
    pairs.  exp writes es pair-slabs directly in fp8 with a -1.5 shift
    (es in (0,448): no row-max pass needed).  In pair-1 chunks, j-tiles
    {4,8,12} compute exp on the DVE instead via a Schraudolph bit-trick:
    uint8_sat(8*log2e*(S*scale-SHIFT) + 55.8) reinterpreted as fp8e4
    (uint8 conversion clamps the softmax tail to +0; softmax averaging
    washes out the +-3% mantissa sawtooth: rel err 5.25e-3 vs 5.24e-3).
  - v_aug column 0 is ones, so PV row 0 is the softmax denominator L.
  - AoA + output run in NATURAL orientation per (head, 128-i-block):
      psum_q[i,128] = qT_blk^T  @ wcq_aug            (2 heads tile-packed)
      psum_f[i,129] = aoU_blk^T @ wca_aug            (aoU rows = [L; ao_un])
    wca_aug row 0 = [0.5*bI | bG | 1]: col 128 receives L^T (a free
    partition-transpose of L), and the bias row receives L*b which the
    per-partition rl = 1/L normalization turns back into b.
      IG = (psum_f[:,0:128] * rl) + psum_q           (recip[128,1] + TS + TT)
      out = (1 + tanh(G/2)) * I'                     (I' prescaled by 0.5)
    This kills the old [1,512]-reciprocal/partition-broadcast norm chains
    (~70us of DVE+GpSimd) and most of the serial half-clock tail; the last
    chunk's finals drain round-robin so their DVE chains pipeline.
  - tanh shares the exp ACT table set: zero table switches after warmup.
  - PSUM: S-pair pool 2x[128,1024] (4 banks) + pv 2x[65,512] (2) + finl
    2x[128,512] slabs (2) = 8 banks exactly.  The finl pool hosts BOTH the
    finals psums and the FILLER projections' psums: the S-pair pool is
    exclusive to the exp pipeline.  A filler unit holding an S-pool slot
    across several steps degrades S to single-buffering and bubbles the
    exp stream ~1us each time (~10us/run); same for the finals DVE chain
    (~5us/chunk when shared).
"""

import numpy as np
from collections import deque
from contextlib import ExitStack

import concourse.bass as bass
import concourse.bacc as bacc
import concourse.tile as tile
from concourse import mybir

B, N, DIM, H, DH = 2, 2048, 1024, 16, 64
HPC = H // 4          # 4 heads per core
INC = HPC * DH        # 256 per-core inner width
KT = DIM // 128       # 8 contraction tiles
NCH = N // 512        # 4 free-dim chunks of 512
JT = N // 128         # 16 j tiles
LAGP = 2              # pv lags exp by LAGP j-tile PAIRS (fp8 DoubleRow)
JP = JT // 2          # 8 j-tile pairs
SHIFT = 1.5           # exp(S*scale - SHIFT) keeps es in fp8e4 range
SCALE = float(DH) ** -0.5
# Schraudolph exp in fp8e4 bit space: bits = round(8*log2e*(S*SCALE - SHIFT)
# + 7*8 - sawtooth centering); reinterpret int8 bits as fp8e4.
EXA = 8.0 * 1.4426950408889634 * SCALE
EXB = 7 * 8 - 8.0 * 1.4426950408889634 * SHIFT - 0.1744
F32 = mybir.dt.float32
BF16 = mybir.dt.bfloat16
FP8 = mybir.dt.float8e4
PM = mybir.MatmulPerfMode
AF = mybir.ActivationFunctionType
ALU = mybir.AluOpType


def build_nc():
    nc = bacc.Bacc(
        "TRN2",
        target_bir_lowering=False,
        debug=False,
        enable_asserts=False,
        num_devices=8,
    )
    # All inputs pre-arranged on host partition-major (dim0 = SBUF partition)
    # and chunk-major so every DMA has >=4KB contiguous per partition line.
    xT_d = nc.dram_tensor("xT", (128, NCH, KT, 512), BF16, kind="ExternalInput").ap()
    xT8_d = nc.dram_tensor("xT8", (128, NCH, KT // 2, 2, 512), FP8,
                           kind="ExternalInput").ap()
    wq_d = nc.dram_tensor("wq", (128, KT, INC), BF16, kind="ExternalInput").ap()
    wkv8_d = nc.dram_tensor("wkv8", (128, KT // 2, 2, 2 * INC), FP8,
                            kind="ExternalInput").ap()
    wcq_d = nc.dram_tensor("wcq", (DH, 2 * DH), BF16, kind="ExternalInput").ap()
    wca_d = nc.dram_tensor("wca", (DH + 1, 2 * DH + 1), BF16, kind="ExternalInput").ap()
    out_d = nc.dram_tensor("out", (N, INC), BF16, kind="ExternalOutput").ap()

    with tile.TileContext(nc) as tc, ExitStack() as ctx:
        consts = ctx.enter_context(tc.tile_pool(name="consts", bufs=1))
        psum = ctx.enter_context(tc.tile_pool(name="psum", bufs=2, space="PSUM"))
        esp = ctx.enter_context(tc.tile_pool(name="es_p", bufs=6))
        rlp = ctx.enter_context(tc.tile_pool(name="rl_p", bufs=4))
        finl = ctx.enter_context(tc.tile_pool(name="finl", bufs=2, space="PSUM"))
        outg_pool = ctx.enter_context(tc.tile_pool(name="outg", bufs=2))
        tpool = ctx.enter_context(tc.tile_pool(name="tanh_p", bufs=2))

        # persistent tensors
        qTp = [consts.tile([128, N], BF16, name=f"qTp{p}") for p in range(2)]
        kTp = [consts.tile([128, N], BF16, name=f"kTp{p}") for p in range(2)]
        # fp8 v, jt-pair interleaved for DoubleRow: [p, jp, h, parity, 80pad]
        v_aug = consts.tile([128, JP, HPC, 2, 80], FP8, name="v_aug")
        nc.vector.memset(v_aug[:, :, :, :, 0:1], 1.0)  # ones col -> L in pv row 0
        aoU = [consts.tile([DH + 1, N], BF16, name=f"aoU{h}") for h in range(HPC)]
        IG = [consts.tile([128, JT, 2 * DH], BF16, name=f"IG{h}") for h in range(HPC)]

        xw = tc.alloc_tile_pool(name="xw", bufs=1)
        wq_sb = xw.tile([128, KT, INC], BF16, name="wq_sb")
        wkv8_sb = xw.tile([128, KT // 2, 2, 2 * INC], FP8, name="wkv8_sb")
        xt_sb = xw.tile([128, NCH, KT, 512], BF16, name="xt_sb")
        xt8_sb = xw.tile([128, NCH, KT // 2, 2, 512], FP8, name="xt8_sb")

        # DMA order = critical path to first exp: the fp8 k/v-projection path
        # first (wk|wv + all of x8), then the bf16 q path.  Few, large,
        # host-side-contiguous transfers (each dma_start costs ~0.6us of
        # sync-engine issue time, and a single queue tops out ~100GB/s).
        nc.sync.dma_start(out=wkv8_sb, in_=wkv8_d)
        nc.sync.dma_start(out=xt8_sb[:, 0], in_=xT8_d[:, 0])
        nc.sync.dma_start(out=wq_sb, in_=wq_d)
        nc.sync.dma_start(out=xt_sb[:, 0], in_=xT_d[:, 0])
        for c in range(1, NCH):
            nc.sync.dma_start(out=xt8_sb[:, c], in_=xT8_d[:, c])
        for c in range(1, NCH):
            nc.sync.dma_start(out=xt_sb[:, c], in_=xT_d[:, c])

        # wcq duplicated on partitions 0:64 and 64:128 so each head's MM1 rhs
        # starts at the same partition as its qTp lhsT slice.
        wcq_sb = consts.tile([128, 2 * DH], BF16, name="wcq_sb")
        nc.sync.dma_start(out=wcq_sb[0:DH, :], in_=wcq_d)
        nc.sync.dma_start(out=wcq_sb[DH:128, :], in_=wcq_d)
        wca_sb = consts.tile([DH + 1, 2 * DH + 1], BF16, name="wca_sb")
        nc.sync.dma_start(out=wca_sb, in_=wca_d)

        # Prefetch the exp/tanh ACT table set (one set covers both).
        warm_sb = consts.tile([128, 1], F32, name="warm_sb")
        nc.vector.memset(warm_sb, 0.25)
        nc.scalar.activation(out=warm_sb, in_=warm_sb, func=AF.Exp)
        nc.scalar.activation(out=warm_sb, in_=warm_sb, func=AF.Tanh)


        ones_sb = consts.tile([128, 1], F32, name="ones_sb")
        nc.vector.memset(ones_sb, 1.0)
        shift_sb = consts.tile([128, 1], F32, name="shift_sb")
        nc.vector.memset(shift_sb, -SHIFT)

        # ---------------- projection units (micro closures) ----------------
        def q_micros(p, c, fill=False):
            cs = slice(c * 512, (c + 1) * 512)
            state = {}

            def mk_mm(k):
                def _mm():
                    if "ps" not in state:
                        state["ps"] = (
                            finl.tile([128, 512], F32, name="ps_q", tag="fin",
                                      bufs=2)
                            if fill else
                            psum.tile([128, 1024], F32, name="ps_q", tag="sp",
                                      bufs=2)
                        )
                    nc.tensor.matmul(
                        state["ps"][:, 0:512],
                        lhsT=wq_sb[:, k, p * 128 : (p + 1) * 128],
                        rhs=xt_sb[:, c, k, :],
                        start=(k == 0),
                        stop=(k == KT - 1),
                    )
                return _mm

            def _copy():
                nc.vector.tensor_copy(out=qTp[p][:, cs], in_=state["ps"][:, 0:512])

            return [mk_mm(k) for k in range(KT)] + [_copy]

        def k_micros(p, c, fill=False):
            cs = slice(c * 512, (c + 1) * 512)
            state = {}

            def mk_mm(kp):
                def _mm():
                    if "ps" not in state:
                        state["ps"] = (
                            finl.tile([128, 512], F32, name="ps_k", tag="fin",
                                      bufs=2)
                            if fill else
                            psum.tile([128, 1024], F32, name="ps_k", tag="sp",
                                      bufs=2)
                        )
                    nc.tensor.matmul(
                        state["ps"][:, 0:512],
                        lhsT=wkv8_sb[:, kp, :, p * 128 : (p + 1) * 128],
                        rhs=xt8_sb[:, c, kp, :, :],
                        start=(kp == 0),
                        stop=(kp == KT // 2 - 1),
                        perf_mode=PM.DoubleRow,
                    )
                return _mm

            def _copy():
                nc.vector.tensor_copy(out=kTp[p][:, cs], in_=state["ps"][:, 0:512])

            return [mk_mm(kp) for kp in range(KT // 2)] + [_copy]

        def v_micros(it, fill=False):
            state = {}

            def mk_mm(kp):
                def _mm():
                    if "ps" not in state:
                        state["ps"] = (
                            finl.tile([128, 512], F32, name="ps_v", tag="fin",
                                      bufs=2)
                            if fill else
                            psum.tile([128, 1024], F32, name="ps_v", tag="sp",
                                      bufs=2)
                        )
                    nc.tensor.matmul(
                        state["ps"][:, 0:INC],
                        lhsT=xt8_sb[:, it // 4, kp, :,
                                    (it % 4) * 128 : (it % 4 + 1) * 128],
                        rhs=wkv8_sb[:, kp, :, INC : 2 * INC],
                        start=(kp == 0),
                        stop=(kp == KT // 2 - 1),
                        perf_mode=PM.DoubleRow,
                    )
                return _mm

            def _copy():
                nc.vector.tensor_copy(
                    out=v_aug[:, it // 2, :, it % 2, 1 : DH + 1],
                    in_=state["ps"][:, 0:INC].rearrange("p (h d) -> p h d", h=HPC),
                )

            return [mk_mm(kp) for kp in range(KT // 2)] + [_copy]

        # ---------------- natural-orientation finals ----------------
        # Per head one [128,257] psum tile (own pool so the fin DVE chain
        # never gates S-tile allocation): q-part 0:128, a-part 128:257.
        def fin_micros(p, ib):
            ibs = slice(ib * 128, (ib + 1) * 128)
            st = {}

            def mk_mm1(hh):
                def _mm():
                    ft = finl.tile([128, 512], F32, name=f"fin{hh}", tag="fin",
                                   bufs=2)
                    st[hh] = ft
                    nc.tensor.matmul(
                        ft[:, 0:128],
                        lhsT=qTp[p][hh * DH : (hh + 1) * DH, ibs],
                        rhs=wcq_sb[hh * DH : (hh + 1) * DH, :],
                        start=True,
                        stop=True,
                        tile_position=(hh * 64, 0),
                    )
                return _mm

            def mk_mm2(hh):
                def _mm():
                    nc.tensor.matmul(
                        st[hh][:, 128:257],
                        lhsT=aoU[2 * p + hh][:, ibs],
                        rhs=wca_sb,
                        start=True,
                        stop=True,
                    )
                return _mm

            def mk_fin(hh):
                def _fin():
                    h = 2 * p + hh
                    ft = st[hh]
                    rl = rlp.tile([128, 1], F32, name="rl", tag="rl")
                    nc.vector.reciprocal(out=rl, in_=ft[:, 256:257])
                    tmp = rlp.tile([128, 128], BF16, name="fa", tag="fa", bufs=2)
                    nc.vector.tensor_scalar(
                        tmp, ft[:, 128:256], rl, None, ALU.mult
                    )
                    nc.vector.tensor_tensor(
                        out=IG[h][:, ib, :], in0=tmp, in1=ft[:, 0:128],
                        op=ALU.add,
                    )
                return _fin

            return [mk_mm1(0), mk_mm1(1), mk_mm2(0), mk_fin(0), mk_mm2(1), mk_fin(1)]

        outg_state = {}

        def sig_micros(p, half):
            """sigmoid-gate + output for heads (2p, 2p+1), i-halves of 1024."""
            def mk_sig(hh):
                def _sig():
                    h = 2 * p + hh
                    if (p, half) not in outg_state:
                        outg_state[(p, half)] = outg_pool.tile(
                            [128, 8, 128], BF16, name="outg", tag="outg"
                        )
                    og = outg_state[(p, half)]
                    hs = slice(half * 8, (half + 1) * 8)
                    t = tpool.tile([128, 8, DH], BF16, name="tnh", tag="tnh")
                    nc.scalar.activation(
                        out=t, in_=IG[h][:, hs, DH : 2 * DH], func=AF.Tanh, scale=0.5
                    )
                    nc.vector.scalar_tensor_tensor(
                        out=og[:, :, hh * DH : (hh + 1) * DH],
                        in0=t,
                        scalar=ones_sb,
                        in1=IG[h][:, hs, 0:DH],
                        op0=ALU.add,
                        op1=ALU.mult,
                    )
                return _sig

            def _dma():
                og = outg_state[(p, half)]
                dv = out_d[half * 1024 : (half + 1) * 1024, p * 128 : (p + 1) * 128]
                nc.sync.dma_start(
                    out=dv.rearrange("(j q) c -> q j c", q=128), in_=og
                )

            return [mk_sig(0), mk_sig(1), _dma]

        # ---------------- filler machinery ----------------
        # Micro-granular: pop one ~120ns micro-closure at a time so filler
        # work never inserts multi-us bursts ahead of the S matmuls.
        fillers = deque()
        emitted = set()
        pending = [None]  # (uid, micros, next_idx)

        def add_unit(uid, micros):
            fillers.append((uid, micros))

        def _step():
            """Emit one micro; return False if nothing left."""
            if pending[0] is None:
                if not fillers:
                    return False
                u, ms = fillers.popleft()
                pending[0] = [u, ms, 0]
            u, ms, i = pending[0]
            ms[i]()
            if i + 1 == len(ms):
                emitted.add(u)
                pending[0] = None
            else:
                pending[0][2] = i + 1
            return True

        def need(uid):
            while uid not in emitted:
                if not _step():
                    raise RuntimeError(f"need({uid}) but queue empty")

        credit = [0.0]

        def pump(ns):
            credit[0] += ns
            while credit[0] > 0.0 and _step():
                credit[0] -= 120.0

        # queue order = need order; deadlines within chunk 0:
        # k0c1@jt4, v0-1@5, v2-3@7, k0c2@8, v4-5@9, v6-7@11, k0c3@12,
        # v8-13@13-15, q0c1@16.  Only k0c0+q0c0 run in the prefix: the exp
        # stream starts as soon as their DMA lands (~2.25MB ahead of it).
        for it in range(0, 2):
            add_unit(("v", it), v_micros(it, fill=True))
        add_unit(("k", 0, 1), k_micros(0, 1, fill=True))
        for it in range(2, 4):
            add_unit(("v", it), v_micros(it, fill=True))
        add_unit(("k", 0, 2), k_micros(0, 2, fill=True))
        for it in range(4, 8):
            add_unit(("v", it), v_micros(it, fill=True))
        add_unit(("k", 0, 3), k_micros(0, 3, fill=True))
        for it in range(8, 14):
            add_unit(("v", it), v_micros(it, fill=True))
        add_unit(("q", 0, 1), q_micros(0, 1, fill=True))
        for it in range(14, 16):
            add_unit(("v", it), v_micros(it, fill=True))
        for c in range(2, NCH):
            add_unit(("q", 0, c), q_micros(0, c, fill=True))
        for c in range(NCH):
            add_unit(("k", 1, c), k_micros(1, c, fill=True))
        for c in range(NCH):
            add_unit(("q", 1, c), q_micros(1, c, fill=True))

        # ---------------- prefix ----------------
        for m in k_micros(0, 0):
            m()
        emitted.add(("k", 0, 0))
        for m in q_micros(0, 0):
            m()
        emitted.add(("q", 0, 0))

        # ---------------- attention ----------------
        def emit_pv(p, jp, pv, es_pairs):
            for hh in range(2):
                nc.tensor.matmul(
                    pv[hh],
                    lhsT=v_aug[:, jp, 2 * p + hh, :, 0 : DH + 1],
                    rhs=es_pairs[jp][:, :, hh * 512 : (hh + 1) * 512],
                    start=(jp == 0),
                    stop=(jp == JP - 1),
                    perf_mode=PM.DoubleRow,
                )

        for p in range(2):
            for c in range(NCH):
                cs = slice(c * 512, (c + 1) * 512)
                pv = [
                    psum.tile([DH + 1, 512], F32, name=f"pv{hh}", tag="pv", bufs=2)
                    for hh in range(2)
                ]
                es_pairs = [None] * JP

                for jt in range(JT):
                    need(("k", p, jt // 4))
                    if jt == 0:
                        need(("q", p, c))
                    jts = slice(jt * 128, (jt + 1) * 128)
                    s = psum.tile([128, 1024], F32, name="s", tag="sp", bufs=2)
                    nc.tensor.matmul(
                        s[:, 0:512],
                        lhsT=kTp[p][0:DH, jts],
                        rhs=qTp[p][0:DH, cs],
                        start=True,
                        stop=True,
                        tile_position=(0, 0),
                    )
                    nc.tensor.matmul(
                        s[:, 512:1024],
                        lhsT=kTp[p][DH:128, jts],
                        rhs=qTp[p][DH:128, cs],
                        start=True,
                        stop=True,
                        tile_position=(64, 0),
                    )
                    jp, par = jt // 2, jt % 2
                    if par == 0:
                        es_pairs[jp] = esp.tile([128, 2, 1024], FP8, name="es",
                                                tag="es")
                    if p == 1 and jt in (4, 8, 12):
                        # Schraudolph exp on DVE (fp8e4 bit-space): frees the
                        # ScalarE stream where it is the pacing engine.
                        # uint8 convert saturates at 0 (clamps the clipped
                        # tail of the softmax to es=+0); max bits ~115 < 126.
                        es8 = es_pairs[jp].bitcast(mybir.dt.uint8)
                        nc.vector.tensor_scalar(
                            es8[:, par, :], s, EXA, EXB, ALU.mult, ALU.add
                        )
                    else:
                        nc.scalar.activation(
                            out=es_pairs[jp][:, par, :], in_=s, func=AF.Exp,
                            scale=SCALE, bias=shift_sb,
                        )
                    if par == 1 and jp >= LAGP:
                        need(("v", 2 * (jp - LAGP) + 1))
                        emit_pv(p, jp - LAGP, pv, es_pairs)
                    pump(330.0 if p == 0 else 430.0)

                for jp in range(JP - LAGP, JP):
                    need(("v", 2 * jp + 1))
                    emit_pv(p, jp, pv, es_pairs)
                for hh in range(2):
                    nc.vector.tensor_copy(out=aoU[2 * p + hh][:, cs], in_=pv[hh])

                # finals for this chunk become filler work
                for ib in range(4 * c, 4 * c + 4):
                    add_unit(("fin", p, ib), fin_micros(p, ib))
                if c % 2 == 1:
                    add_unit(("sig", p, c // 2), sig_micros(p, c // 2))

            if p == 0:
                xw.release()

        # ---------------- tail: round-robin the remaining units so their
        # PE matmuls pipeline under the DVE chains (sig units last: they
        # read IG written by the fin units).
        rem = []
        if pending[0] is not None:
            u, ms, i = pending[0]
            rem.append([u, ms, i])
            pending[0] = None
        while fillers:
            u, ms = fillers.popleft()
            rem.append([u, ms, 0])
        sigs = [r for r in rem if r[0][0] == "sig"]
        rr = [r for r in rem if r[0][0] != "sig"]
        prog = True
        while prog:
            prog = False
            for r in rr:
                if r[2] < len(r[1]):
                    r[1][r[2]]()
                    r[2] += 1
                    prog = True
        for r in sigs:
            for i in range(r[2], len(r[1])):
                r[1][i]()
    nc.compile()
    return nc


_NC_CACHE = None


def _get_nc():
    global _NC_CACHE
    if _NC_CACHE is None:
        _NC_CACHE = build_nc()
    return _NC_CACHE


def make_in_maps(x, Wq, Wkv, Wq_out, Wattn_out, out_bias, Wq_gate, Wattn_gate,
                 gate_bias):
    import ml_dtypes

    bf16 = ml_dtypes.bfloat16
    # wcq_aug [64,128] = [0.5*Wq_out^T | Wq_gate^T]
    wcq = np.ascontiguousarray(
        np.concatenate([0.5 * Wq_out.T, Wq_gate.T], axis=1), dtype=bf16
    )
    # wca_aug [65,129]: row0 = [0.5*bI | bG | 1]; rows1:65 = [0.5*Wao^T | Wag^T | 0]
    wca = np.zeros((DH + 1, 2 * DH + 1), dtype=np.float32)
    wca[0, 0:DH] = 0.5 * out_bias.reshape(-1)
    wca[0, DH : 2 * DH] = gate_bias.reshape(-1)
    wca[0, 2 * DH] = 1.0
    wca[1:, 0:DH] = 0.5 * Wattn_out.T
    wca[1:, DH : 2 * DH] = Wattn_gate.T
    wca = np.ascontiguousarray(wca).astype(bf16)
    f8 = ml_dtypes.float8_e4m3
    Wk = Wkv[:, : H * DH]
    Wv = Wkv[:, H * DH :]
    # x[b].T is (DIM, N): [k*128+p, c*512+j].  Partition-major chunk-major:
    # xT   (128, NCH, KT, 512):      [p, c, k, j]
    # xT8  (128, NCH, KT//2, 2, 512) [p, c, kp, par, j] with k = 2*kp+par
    xTb = [
        np.ascontiguousarray(
            x[b].T.reshape(KT, 128, NCH, 512).transpose(1, 2, 0, 3)
        )
        for b in range(B)
    ]
    xT = [t.astype(bf16) for t in xTb]
    xT8 = [
        np.ascontiguousarray(t.reshape(128, NCH, KT // 2, 2, 512)).astype(f8)
        for t in xTb
    ]
    in_maps = []
    for c in range(8):
        b, hg = c // 4, c % 4
        cols = slice(hg * INC, (hg + 1) * INC)
        # weights partition-major: wq (128, KT, INC), wkv8 (128, KT//2, 2, 512)
        wq_pm = np.ascontiguousarray(
            Wq[:, cols].reshape(KT, 128, INC).transpose(1, 0, 2)
        ).astype(bf16)
        wkv_c = np.concatenate([Wk[:, cols], Wv[:, cols]], axis=1)
        wkv8_pm = np.ascontiguousarray(
            wkv_c.reshape(KT // 2, 2, 128, 2 * INC).transpose(2, 0, 1, 3)
        ).astype(f8)
        in_maps.append(
            {
                "xT": xT[b],
                "xT8": xT8[b],
                "wq": wq_pm,
                "wkv8": wkv8_pm,
                "wcq": wcq,
                "wca": wca,
            }
        )
    return in_maps


def assemble_output(results):
    out = np.empty((B, N, H * DH), dtype=np.float32)
    for c in range(8):
        b, hg = c // 4, c % 4
        out[b, :, hg * INC : (hg + 1) * INC] = results[c]["out"].astype(np.float32)
    return out


def kernel(**inputs):
    from concourse.bass_utils import run_bass_kernel_spmd

    inputs = {k: np.asarray(v, dtype=np.float32) for k, v in inputs.items()}
    nc = _get_nc()
    in_maps = make_in_maps(**inputs)
    res = run_bass_kernel_spmd(nc, in_maps, core_ids=list(range(8)))
    return assemble_output(res.results)
